# revision 1
# baseline (speedup 1.0000x reference)
"""Multi-head attention (B=4, T=S=2048, E=1024, H=16) on 8 trn2 NeuronCores.

Sharding: core c handles batch b = c // 2 and head-half hh = c % 2
(8 of 16 heads).  Each core computes its heads' Q/K/V projections,
attention, and a partial output projection (contraction over its 512
e-dims).  The host sums the two partial outputs per batch and adds bo.

On-chip layout is "transposed activations": scores are computed as
scores.T = kp @ qp.T  ([s, t], s on partitions), softmax denominators
come from an extra ones-column in the PV matmul (M=65), and the output
projection consumes ctx.T directly, producing out.T per core.
Activations/weights are transposed on-chip via PE (tensor-engine)
transposes after a DVE fp32->bf16 cast.
"""

import numpy as np

import concourse.bass as bass
import concourse.mybir as mybir
import concourse.tile as tile
from concourse.bass_utils import run_bass_kernel_spmd
from concourse.masks import make_identity

F32 = mybir.dt.float32
BF16 = mybir.dt.bfloat16

B, T, E = 4, 2048, 1024
H = 16  # global heads
HL = 8  # heads per core (local)
HD = 64  # head dim
EL = HL * HD  # 512, e-dims per core
N_CORES = 8

_CACHED = {}


def legalize_waits(nc, cap=1):
    """Hoist semaphore waits so no instruction carries more than `cap`.

    The cayman 64B ISA instruction format has a single wait slot
    (NEURON_ISA_TPB_EVENTS); this container's walrus rejects instructions
    with more attached waits ("Too many sync wait commands").  Tile's sem
    assignment freely attaches several, so we split the excess onto
    standalone InstEventSemaphore carriers (exactly what raw-bass
    wait_ge emits) on the same engine, immediately before.
    """
    import bass_rust

    # Pass 1: statically-known final value of every semaphore (sum of all
    # attached increments) — needed to replace the tail RANGE_CLEAR (an
    # InstISA opcode this walrus can't codegen) with sem-dec updates.
    totals = {}
    names = {}
    for f in nc.m.functions:
        for bb in f.blocks:
            for ins in bb.instructions:
                si = ins.sync_info
                if si is None:
                    continue
                for u in si.on_update or []:
                    if u.sync_type == "semaphore":
                        sign = 1 if u.update_mode in ("sem-inc", "sem-add-imm") else -1
                        totals[u.id] = totals.get(u.id, 0) + sign * u.update_value
                        names[u.id] = u.ant_name

    n = 0
    for f in nc.m.functions:
        for bb in f.blocks:
            insts = bb.instructions
            out = []
            changed = False
            for ins in insts:
                if type(ins).__name__ == "InstISA" and "RANGE_CLEAR" in str(ins):
                    import re

                    m = re.search(r"range_first=(\d+) range_last=(\d+)", str(ins))
                    first, last = int(m.group(1)), int(m.group(2))
                    for sid in range(first, last + 1):
                        tot = totals.get(sid, 0)
                        if tot == 0:
                            continue
                        ev = mybir.InstEventSemaphore(name=f"I-LC{n}", ins=[], outs=[])
                        n += 1
                        ev.engine = ins.engine
                        ev.sync_info = bass_rust.SyncInfo(
                            on_wait=[],
                            on_update=[
                                bass_rust.SyncUpdate(
                                    sync_type="semaphore",
                                    id=sid,
                                    ant_name=names.get(sid, f"sem{sid}"),
                                    update_mode="sem-sub-imm",
                                    update_value=tot,
                                    update_reg=None,
                                )
                            ],
                        )
                        out.append(ev)
                    changed = True
                    continue
                si = ins.sync_info
                ws = list(si.on_wait) if (si is not None and si.on_wait) else []
                if len(ws) > cap:
                    for w in ws[: len(ws) - cap]:
                        ev = mybir.InstEventSemaphore(
                            name=f"I-LW{n}", ins=[], outs=[]
                        )
                        n += 1
                        ev.engine = ins.engine
                        ev.sync_info = bass_rust.SyncInfo(
                            on_wait=[w], on_update=[]
                        )
                        out.append(ev)
                    si.on_wait = ws[len(ws) - cap :]
                    changed = True
                out.append(ins)
            if changed:
                insts[:] = out
    return n


def build_program():
    nc = bass.Bass()

    qd = nc.declare_dram_parameter("q", [T, E], F32, isOutput=False)
    kd = nc.declare_dram_parameter("k", [T, E], F32, isOutput=False)
    vd = nc.declare_dram_parameter("v", [T, E], F32, isOutput=False)
    wqd = nc.declare_dram_parameter("wq", [EL, E], F32, isOutput=False)
    wkd = nc.declare_dram_parameter("wk", [EL, E], F32, isOutput=False)
    wvd = nc.declare_dram_parameter("wv", [EL, E], F32, isOutput=False)
    wod = nc.declare_dram_parameter("wo", [E, EL], F32, isOutput=False)
    bqd = nc.declare_dram_parameter("bq", [EL], F32, isOutput=False)
    bkd = nc.declare_dram_parameter("bk", [EL], F32, isOutput=False)
    bvd = nc.declare_dram_parameter("bv", [EL], F32, isOutput=False)
    outd = nc.declare_dram_parameter("outT", [E, T], F32, isOutput=True)

    with tile.TileContext(nc, pool_alloc_mode="queue") as tc:
        with (
            tc.tile_pool(name="singles", bufs=1) as singles,
            tc.tile_pool(name="stage", bufs=2) as stage,
            tc.tile_pool(name="xt", bufs=1) as xtp,
            tc.tile_pool(name="acts", bufs=1) as acts,
            tc.tile_pool(name="pt", bufs=6) as ptp,
            tc.tile_pool(name="norm", bufs=4) as normp,
            tc.tile_pool(name="osb", bufs=4) as osbp,
            tc.tile_pool(name="dram", bufs=2, space="DRAM") as dramp,
            tc.tile_pool(name="proj_ps", bufs=2, space="PSUM") as proj_ps,
            tc.tile_pool(name="sc_ps", bufs=2, space="PSUM") as sc_ps,
            tc.tile_pool(name="ctx_ps", bufs=2, space="PSUM") as ctx_ps,
        ):
            # ---------------- prologue: weights / biases / consts ----------
            ident = singles.tile([128, 128], BF16)
            make_identity(nc, ident)

            # Transposed bf16 weights:
            #   WqT[p, c, m] = Wq_c[m, c*128 + p]   (c,p) = e in [0,1024)
            wqT = singles.tile([128, 8, EL], BF16)
            wkT = singles.tile([128, 8, EL], BF16)
            wvT = singles.tile([128, 8, EL], BF16)
            # WoT[p, c, o] = Wo_c[o, c*128 + p]     (c,p) = local e in [0,512)
            woT = singles.tile([128, 4, E], BF16)

            def load_cast(xd, nrows):
                """Load [nrows, ncols] f32 DRAM -> bf16 SBUF natural tile
                [128, nrows//128, ncols] (cast happens in the SWDGE DMA).
                Chunked by 512 rows so downstream PE transposes can start
                before the whole tensor has landed."""
                a = nrows // 128
                ncols = xd.shape[1]
                xb = stage.tile([128, a, ncols], BF16, tag="stage_b")
                step = min(4, a)
                for blk in range(0, a, step):
                    nc.gpsimd.dma_start(
                        out=xb[:, blk : blk + step, :],
                        in_=xd[blk * 128 : (blk + step) * 128, :].rearrange(
                            "(a p) e -> p a e", p=128
                        ),
                    )
                return xb

            def pe_transpose(dst, src, a_chunks, e_chunks):
                """dst[p, e, a*128 + t] = src[t(p), a, e*128 + p] via PE
                transposes; dst is [128, e_chunks, a_chunks*128]."""
                for e in range(e_chunks):
                    # stack the a_chunks transposes of e-chunk e into one
                    # psum tile, then copy out in one DVE op
                    n = a_chunks * 128
                    tr = sc_ps.tile([128, n], BF16, tag="sc")
                    for a in range(a_chunks):
                        nc.tensor.transpose(
                            tr[:, a * 128 : (a + 1) * 128],
                            src[:, a, e * 128 : (e + 1) * 128],
                            ident,
                        )
                    nc.vector.tensor_copy(out=dst[:, e, :], in_=tr)

            # wv first: the v projection is the first consumer of any weight,
            # so its transpose should be ready earliest.
            for wd, wT in ((wvd, wvT), (wqd, wqT), (wkd, wkT)):
                wb = load_cast(wd, EL)
                pe_transpose(wT, wb, a_chunks=4, e_chunks=8)
            wob = load_cast(wod, E)
            pe_transpose(woT, wob, a_chunks=8, e_chunks=4)

            # biases: bq_sb[p, c] = bq[c*128 + p]
            bq_sb = singles.tile([128, 4], F32)
            bk_sb = singles.tile([128, 4], F32)
            nc.gpsimd.dma_start(out=bq_sb, in_=bqd.rearrange("(c p) -> p c", p=128))
            nc.gpsimd.dma_start(out=bk_sb, in_=bkd.rearrange("(c p) -> p c", p=128))
            bv_sb = singles.tile([1, EL], BF16)
            nc.gpsimd.dma_start(out=bv_sb, in_=bvd.rearrange("(o e) -> o e", o=1))
            ones_col = singles.tile([1, 128], BF16)
            nc.vector.memset(ones_col, 1.0)
            ones64b = singles.tile([1, 64], BF16)
            nc.vector.memset(ones64b, 1.0)

            # ---------------- projections --------------------------------
            # qpT[p, j, t] = qp[t, j*128 + p]  (pair j: head 2j at p<64)
            qpT = acts.tile([128, 4, T], BF16)
            kpT = acts.tile([128, 4, T], BF16)
            # vp_ext[p, s, h*65 + d] = vp[s*128 + p, h*64 + d]; col h*65+64 = 1.0
            vp_ext = acts.tile([128, 16, HL * 65], BF16)

            def load_xT(xd):
                """x [T, E] f32 DRAM -> xT[p, c, t] = x[t, c*128 + p] bf16."""
                xT = xtp.tile([128, 8, T], BF16, tag="xT")
                for blk in range(4):
                    xb = load_cast(xd[blk * 512 : (blk + 1) * 512, :], 512)
                    # xb[pt, a, e]: t = blk*512 + a*128 + pt
                    for e in range(8):
                        tr = sc_ps.tile([128, 512], BF16, tag="sc")
                        for a in range(4):
                            nc.tensor.transpose(
                                tr[:, a * 128 : (a + 1) * 128],
                                xb[:, a, e * 128 : (e + 1) * 128],
                                ident,
                            )
                        nc.vector.tensor_copy(
                            out=xT[:, e, blk * 512 : (blk + 1) * 512], in_=tr
                        )
                return xT

            # k and q first: scores (and the ACT exp stream) depend only on
            # kpT/qpT, while vp_ext is consumed per-s-chunk by PV later.
            for xd, xpT, b_sb, wT in ((kd, kpT, bk_sb, wkT), (qd, qpT, bq_sb, wqT)):
                xT = load_xT(xd)
                for c in range(4):
                    for tb in range(4):
                        ps = proj_ps.tile([128, 512], F32, tag="proj")
                        for e in range(8):
                            nc.tensor.matmul(
                                ps,
                                lhsT=wT[:, e, c * 128 : (c + 1) * 128],
                                rhs=xT[:, e, tb * 512 : (tb + 1) * 512],
                                start=(e == 0),
                                stop=(e == 7),
                            )
                        nc.vector.tensor_scalar_add(
                            out=xpT[:, c, tb * 512 : (tb + 1) * 512],
                            in0=ps,
                            scalar1=b_sb[:, c : c + 1],
                        )

            vT = load_xT(vd)
            for s in range(16):
                ps = proj_ps.tile([128, 512], F32, tag="proj")
                for e in range(8):
                    nc.tensor.matmul(
                        ps,
                        lhsT=vT[:, e, s * 128 : (s + 1) * 128],
                        rhs=wvT[:, e, :],
                        start=(e == 0),
                        stop=False,
                    )
                # += ones ⊗ bv  (bias along the free dim)
                nc.tensor.matmul(ps, lhsT=ones_col, rhs=bv_sb, start=False, stop=True)
                nc.vector.memset(vp_ext[:, s, :], 1.0)
                nc.vector.tensor_copy(
                    out=vp_ext[:, s, :].rearrange("p (h x) -> p h x", x=65)[:, :, 0:64],
                    in_=ps.rearrange("p (h d) -> p h d", d=64),
                )

            # ---------------- attention ----------------------------------
            # ctxn[p, j, t] = ctx[t, j*128 + p] / denom
            ctxn = acts.tile([128, 4, T], BF16)

            for tb in range(4):
                tsl = slice(tb * 512, (tb + 1) * 512)
                for j in range(4):
                    hA, hB = 2 * j, 2 * j + 1
                    ctx_a = ctx_ps.tile([65, 512], F32, tag="ctx")
                    ctx_b = ctx_ps.tile([65, 512], F32, tag="ctx")
                    for s in range(16):
                        ssl = slice(s * 128, (s + 1) * 128)
                        sc = sc_ps.tile([128, 1024], F32, tag="sc")
                        # scores.T tiles, row-packed pair (K=64 each)
                        nc.tensor.matmul(
                            sc[:, 0:512],
                            lhsT=kpT[0:64, j, ssl],
                            rhs=qpT[0:64, j, tsl],
                            start=True,
                            stop=True,
                        )
                        nc.tensor.matmul(
                            sc[:, 512:1024],
                            lhsT=kpT[64:128, j, ssl],
                            rhs=qpT[64:128, j, tsl],
                            start=True,
                            stop=True,
                        )
                        pt = ptp.tile([128, 1024], BF16, tag="pt")
                        nc.scalar.activation(
                            out=pt,
                            in_=sc,
                            func=mybir.ActivationFunctionType.Exp,
                            scale=0.125,
                        )
                        nc.tensor.matmul(
                            ctx_a,
                            lhsT=vp_ext[:, s, hA * 65 : hA * 65 + 65],
                            rhs=pt[:, 0:512],
                            start=(s == 0),
                            stop=(s == 15),
                        )
                        nc.tensor.matmul(
                            ctx_b,
                            lhsT=vp_ext[:, s, hB * 65 : hB * 65 + 65],
                            rhs=pt[:, 512:1024],
                            start=(s == 0),
                            stop=(s == 15),
                        )
                    # normalize: row 64 of ctx_* holds the denominators
                    # Drain ctx PSUM to SBUF right away so the next unit's PV
                    # can start; normalize off the critical path from SBUF.
                    ctxu = normp.tile([65, 1024], F32, tag="ctxu", bufs=2)
                    nc.vector.tensor_copy(out=ctxu[:, 0:512], in_=ctx_a)
                    nc.vector.tensor_copy(out=ctxu[:, 512:1024], in_=ctx_b)
                    # reciprocal of the denominators, then broadcast across
                    # 64 partitions with a tiny fp32 outer-product matmul
                    recf = normp.tile([1, 1024], F32, tag="recf", bufs=2)
                    nc.vector.reciprocal(out=recf[:, 0:512], in_=ctxu[64:65, 0:512])
                    nc.vector.reciprocal(
                        out=recf[:, 512:1024], in_=ctxu[64:65, 512:1024]
                    )
                    recb = normp.tile([1, 1024], BF16, tag="recb", bufs=2)
                    nc.vector.tensor_copy(out=recb, in_=recf)
                    # broadcast across 64 partitions via bf16 outer-product
                    # matmuls in the proj pool (keeps sc ping-pong slots free)
                    bc_a = proj_ps.tile([64, 512], F32, tag="proj")
                    bc_b = proj_ps.tile([64, 512], F32, tag="proj")
                    nc.tensor.matmul(
                        bc_a, lhsT=ones64b, rhs=recb[:, 0:512],
                        start=True, stop=True,
                    )
                    nc.tensor.matmul(
                        bc_b, lhsT=ones64b, rhs=recb[:, 512:1024],
                        start=True, stop=True,
                    )
                    bc_sb = normp.tile([64, 1024], BF16, tag="bc", bufs=2)
                    nc.vector.tensor_copy(out=bc_sb[:, 0:512], in_=bc_a)
                    nc.vector.tensor_copy(out=bc_sb[:, 512:1024], in_=bc_b)
                    nc.vector.tensor_mul(
                        out=ctxn[0:64, j, tsl],
                        in0=ctxu[0:64, 0:512],
                        in1=bc_sb[:, 0:512],
                    )
                    ctxn_b = normp.tile([64, 512], BF16, tag="ctxnb", bufs=2)
                    nc.vector.tensor_mul(
                        out=ctxn_b, in0=ctxu[0:64, 512:1024], in1=bc_sb[:, 512:1024]
                    )
                    nc.sync.dma_start(out=ctxn[64:128, j, tsl], in_=ctxn_b)

                # ---------------- output projection for this t-block -----
                for o in range(8):
                    ps = proj_ps.tile([128, 512], F32, tag="proj")
                    for c in range(4):
                        nc.tensor.matmul(
                            ps,
                            lhsT=woT[:, c, o * 128 : (o + 1) * 128],
                            rhs=ctxn[:, c, tsl],
                            start=(c == 0),
                            stop=(c == 3),
                        )
                    osb = osbp.tile([128, 512], F32, tag="osb")
                    nc.vector.tensor_copy(out=osb, in_=ps)
                    nc.sync.dma_start(
                        out=outd[o * 128 : (o + 1) * 128, tsl], in_=osb
                    )

    legalize_waits(nc)
    return nc


def _make_in_maps(inputs):
    q, k, v = inputs["q"], inputs["k"], inputs["v"]
    in_maps = []
    for c in range(N_CORES):
        b, hh = c // 2, c % 2
        esl = slice(hh * EL, (hh + 1) * EL)
        in_maps.append(
            {
                "q": np.ascontiguousarray(q[b], dtype=np.float32),
                "k": np.ascontiguousarray(k[b], dtype=np.float32),
                "v": np.ascontiguousarray(v[b], dtype=np.float32),
                "wq": np.ascontiguousarray(inputs["Wq"][esl], dtype=np.float32),
                "wk": np.ascontiguousarray(inputs["Wk"][esl], dtype=np.float32),
                "wv": np.ascontiguousarray(inputs["Wv"][esl], dtype=np.float32),
                "wo": np.ascontiguousarray(inputs["Wo"][:, esl], dtype=np.float32),
                "bq": np.ascontiguousarray(inputs["bq"][esl], dtype=np.float32),
                "bk": np.ascontiguousarray(inputs["bk"][esl], dtype=np.float32),
                "bv": np.ascontiguousarray(inputs["bv"][esl], dtype=np.float32),
            }
        )
    return in_maps


def _gather(results, bo):
    out = np.empty((B, T, E), dtype=np.float32)
    for b in range(B):
        acc = results[2 * b]["outT"].T + results[2 * b + 1]["outT"].T
        out[b] = acc + bo[None, :]
    return out


def run(inputs, **spmd_kwargs):
    if "nc" not in _CACHED:
        _CACHED["nc"] = build_program()
    nc = _CACHED["nc"]
    in_maps = _make_in_maps(inputs)
    res = run_bass_kernel_spmd(nc, in_maps, core_ids=list(range(N_CORES)), **spmd_kwargs)
    out = _gather(res.results, np.asarray(inputs["bo"], dtype=np.float32))
    return out, res


def kernel(**inputs) -> np.ndarray:
    out, _ = run(inputs)
    return out



# revision 23
# speedup vs baseline: 1.2712x; 1.2712x over previous
"""Multi-head attention (B=4, T=S=2048, E=1024, H=16) on 8 trn2 NeuronCores.

Sharding: core c handles batch b = c // 2 and head-half hh = c % 2
(8 of 16 heads).  Each core computes its heads' Q/K/V projections,
attention, and a partial output projection (contraction over its 512
e-dims).  The host sums the two partial outputs per batch and adds bo.

Key layout choices (v2):
 - The host passes PRE-TRANSPOSED activations and weights (q.T, k.T,
   v.T, Wq_slice.T, ...), so no on-chip transposes of x or W are
   needed: DMA loads land directly in the [e_in partitions, t] layout
   the projections consume (f32->bf16 cast in the SWDGE DMA).
 - scores.T = kp @ qp.T is computed per head as [s, t] tiles
   (s on partitions), exp'd on ACT into bf16 pt tiles.
 - PV runs in the "natural" orientation: ctx[t, hd] = sum_s
   pt[s, t] * vp[s, hd], i.e. lhsT = pt (stationary), rhs = vp
   with an extra ones-column producing the softmax denominator in
   column 64.  Output columns per matmul are 65 instead of 512,
   which is ~2x fewer PE cycles for the PV stage.
 - Normalization is a per-partition (per-t) reciprocal multiply on
   DVE, no cross-partition broadcast needed.
 - ctx is transposed back (PE transposes) only for the tiny
   [2048 x 512] normalized context, feeding the output projection.
"""

import numpy as np

import concourse.bass as bass
import concourse.mybir as mybir
import concourse.tile as tile
from concourse.bass_utils import run_bass_kernel_spmd
from concourse.masks import make_identity

F32 = mybir.dt.float32
BF16 = mybir.dt.bfloat16

B, T, E = 4, 2048, 1024
H = 16  # global heads
HL = 8  # heads per core (local)
HD = 64  # head dim
EL = HL * HD  # 512, e-dims per core
N_CORES = 8

_CACHED = {}


def legalize_waits(nc, cap=1):
    """Hoist semaphore waits so no instruction carries more than `cap`.

    The cayman 64B ISA instruction format has a single wait slot
    (NEURON_ISA_TPB_EVENTS); this container's walrus rejects instructions
    with more attached waits ("Too many sync wait commands").  Tile's sem
    assignment freely attaches several, so we split the excess onto
    standalone InstEventSemaphore carriers (exactly what raw-bass
    wait_ge emits) on the same engine, immediately before.
    """
    import bass_rust

    # Pass 1: statically-known final value of every semaphore (sum of all
    # attached increments) — needed to replace the tail RANGE_CLEAR (an
    # InstISA opcode this walrus can't codegen) with sem-dec updates.
    totals = {}
    names = {}
    for f in nc.m.functions:
        for bb in f.blocks:
            for ins in bb.instructions:
                si = ins.sync_info
                if si is None:
                    continue
                for u in si.on_update or []:
                    if u.sync_type == "semaphore":
                        sign = 1 if u.update_mode in ("sem-inc", "sem-add-imm") else -1
                        totals[u.id] = totals.get(u.id, 0) + sign * u.update_value
                        names[u.id] = u.ant_name

    n = 0
    for f in nc.m.functions:
        for bb in f.blocks:
            insts = bb.instructions
            out = []
            changed = False
            for ins in insts:
                if type(ins).__name__ == "InstISA" and "RANGE_CLEAR" in str(ins):
                    import re

                    m = re.search(r"range_first=(\d+) range_last=(\d+)", str(ins))
                    first, last = int(m.group(1)), int(m.group(2))
                    for sid in range(first, last + 1):
                        tot = totals.get(sid, 0)
                        if tot == 0:
                            continue
                        ev = mybir.InstEventSemaphore(name=f"I-LC{n}", ins=[], outs=[])
                        n += 1
                        ev.engine = ins.engine
                        ev.sync_info = bass_rust.SyncInfo(
                            on_wait=[],
                            on_update=[
                                bass_rust.SyncUpdate(
                                    sync_type="semaphore",
                                    id=sid,
                                    ant_name=names.get(sid, f"sem{sid}"),
                                    update_mode="sem-sub-imm",
                                    update_value=tot,
                                    update_reg=None,
                                )
                            ],
                        )
                        out.append(ev)
                    changed = True
                    continue
                si = ins.sync_info
                ws = list(si.on_wait) if (si is not None and si.on_wait) else []
                if len(ws) > cap:
                    for w in ws[: len(ws) - cap]:
                        ev = mybir.InstEventSemaphore(
                            name=f"I-LW{n}", ins=[], outs=[]
                        )
                        n += 1
                        ev.engine = ins.engine
                        ev.sync_info = bass_rust.SyncInfo(
                            on_wait=[w], on_update=[]
                        )
                        out.append(ev)
                    si.on_wait = ws[len(ws) - cap :]
                    changed = True
                out.append(ins)
            if changed:
                insts[:] = out
    return n


def build_program():
    nc = bass.Bass()

    # Activations/weights arrive pre-transposed AND pre-cast to bf16 on the
    # host: halves the DMA bytes and removes the cast from the DMA path.
    qtd = nc.declare_dram_parameter("qT", [E, T], BF16, isOutput=False)
    ktd = nc.declare_dram_parameter("kT", [E, T], BF16, isOutput=False)
    vtd = nc.declare_dram_parameter("vT", [E, T], BF16, isOutput=False)
    wqtd = nc.declare_dram_parameter("wqT", [E, EL], BF16, isOutput=False)
    wktd = nc.declare_dram_parameter("wkT", [E, EL], BF16, isOutput=False)
    wvtd = nc.declare_dram_parameter("wvT", [E, EL], BF16, isOutput=False)
    wotd = nc.declare_dram_parameter("woT", [EL, E], BF16, isOutput=False)
    bqd = nc.declare_dram_parameter("bq", [EL], F32, isOutput=False)
    bkd = nc.declare_dram_parameter("bk", [EL], F32, isOutput=False)
    bvd = nc.declare_dram_parameter("bv", [EL], F32, isOutput=False)
    outd = nc.declare_dram_parameter("outT", [E, T], F32, isOutput=True)

    with tile.TileContext(nc, pool_alloc_mode="queue") as tc:
        with (
            tc.tile_pool(name="singles", bufs=1) as singles,
            tc.tile_pool(name="xin", bufs=3) as xin,
            tc.tile_pool(name="acts", bufs=1) as acts,
            tc.tile_pool(name="pt", bufs=1) as ptp,
            tc.tile_pool(name="norm", bufs=8) as normp,
            tc.tile_pool(name="osb", bufs=4) as osbp,
            tc.tile_pool(name="proj_ps", bufs=2, space="PSUM") as proj_ps,
            tc.tile_pool(name="sc_ps", bufs=2, space="PSUM") as sc_ps,
            tc.tile_pool(name="ctx_ps", bufs=1, space="PSUM") as ctx_ps,
        ):
            # ---------------- prologue: weights / biases / consts ----------
            ident = singles.tile([128, 128], BF16)
            make_identity(nc, ident)

            # Transposed bf16 weights, loaded directly (host pre-transposed):
            #   wqT[p, c, o] = Wq_c[o, c*128 + p]   (c,p) = e_in in [0,1024)
            wqT = singles.tile([128, 8, EL], BF16)
            wkT = singles.tile([128, 8, EL], BF16)
            wvT = singles.tile([128, 8, EL], BF16)
            # woT[p, c, o] = Wo_c[o, c*128 + p]     (c,p) = local e in [0,512)
            woT = singles.tile([128, 4, E], BF16)

            # k-path DMAs first: the whole k projection gates attention.
            nc.gpsimd.dma_start(
                out=wkT, in_=wktd.rearrange("(c p) o -> p c o", p=128)
            )
            bq_sb = singles.tile([128, 4], F32)
            bk_sb = singles.tile([128, 4], F32)
            nc.gpsimd.dma_start(out=bk_sb, in_=bkd.rearrange("(c p) -> p c", p=128))
            nc.gpsimd.dma_start(out=bq_sb, in_=bqd.rearrange("(c p) -> p c", p=128))
            ones_col = singles.tile([1, 128], BF16)
            nc.vector.memset(ones_col, 1.0)
            bv_sb = singles.tile([1, EL], BF16)

            # ---------------- activations / projections --------------------
            # qpT[p, j, t] = qp[t, j*128 + p]  (pair j: head 2j at p<64)
            qpT = acts.tile([128, 4, T], BF16)
            kpT = acts.tile([128, 4, T], BF16)
            # vp_ext[p, s, h*65 + d] = vp[s*128 + p, h*64 + d]; col h*65+64 = 1
            vp_ext = acts.tile([128, 16, HL * 65], BF16)

            def load_x_chunk(xd, tb):
                """x.T [E, T] bf16 DRAM chunk tb -> bf16 SBUF [128, 8, 512]."""
                xc = xin.tile([128, 8, 512], BF16, tag="xin")
                nc.gpsimd.dma_start(
                    out=xc,
                    in_=xd[:, tb * 512 : (tb + 1) * 512].rearrange(
                        "(c p) t -> p c t", p=128
                    ),
                )
                return xc

            def proj_qk_group(xc, xpT, b_sb, wT, c, tb):
                """One (e_out chunk c, t-block tb) projection psum group."""
                ps = proj_ps.tile([128, 512], F32, tag="proj")
                for e in range(8):
                    nc.tensor.matmul(
                        ps,
                        lhsT=wT[:, e, c * 128 : (c + 1) * 128],
                        rhs=xc[:, e, :],
                        start=(e == 0),
                        stop=(e == 7),
                    )
                nc.vector.tensor_scalar_add(
                    out=xpT[:, c, tb * 512 : (tb + 1) * 512],
                    in0=ps,
                    scalar1=b_sb[:, c : c + 1],
                )

            def proj_qk_chunk(xc, xpT, b_sb, wT, tb):
                """One t-block of the q/k projection: fills xpT[:, :, tsl]."""
                for c in range(4):
                    proj_qk_group(xc, xpT, b_sb, wT, c, tb)

            def proj_v_schunk(vc, s):
                """One 128-row s-chunk of the v projection -> vp_ext[:, s, :].

                vc is the loaded vT chunk covering s; u = s % 4 indexes the
                128-slice within it.
                """
                u = s % 4
                ps = proj_ps.tile([128, 512], F32, tag="proj")
                for e in range(8):
                    nc.tensor.matmul(
                        ps,
                        lhsT=vc[:, e, u * 128 : (u + 1) * 128],
                        rhs=wvT[:, e, :],
                        start=(e == 0),
                        stop=False,
                    )
                # += ones ⊗ bv  (bias along the free dim)
                nc.tensor.matmul(ps, lhsT=ones_col, rhs=bv_sb, start=False, stop=True)
                nc.vector.memset(vp_ext[:, s, :], 1.0)
                nc.vector.tensor_copy(
                    out=vp_ext[:, s, :].rearrange("p (h x) -> p h x", x=65)[
                        :, :, 0:64
                    ],
                    in_=ps.rearrange("p (h d) -> p h d", d=64),
                )

            # k is projected c-row-wise (c = head-pair j): attention unit
            # (j, tb) only needs row c=j of kpT, so unit (0, 0) can start
            # right after row 0 — rows 1..3 are spread into tb0's units.
            # kT is loaded as one full tile (each c-row contracts over all T).
            ktf = xin.tile([128, 8, T], BF16, tag="ktf", bufs=1)
            nc.gpsimd.dma_start(
                out=ktf[:, :, 0:512],
                in_=ktd[:, 0:512].rearrange("(c p) t -> p c t", p=128),
            )
            nc.gpsimd.dma_start(
                out=ktf[:, :, 512:1024],
                in_=ktd[:, 512:1024].rearrange("(c p) t -> p c t", p=128),
            )
            nc.gpsimd.dma_start(
                out=wqT, in_=wqtd.rearrange("(c p) o -> p c o", p=128)
            )
            qc = load_x_chunk(qtd, 0)
            nc.gpsimd.dma_start(
                out=ktf[:, :, 1024:1536],
                in_=ktd[:, 1024:1536].rearrange("(c p) t -> p c t", p=128),
            )
            nc.gpsimd.dma_start(
                out=ktf[:, :, 1536:2048],
                in_=ktd[:, 1536:2048].rearrange("(c p) t -> p c t", p=128),
            )
            nc.gpsimd.dma_start(
                out=wvT, in_=wvtd.rearrange("(c p) o -> p c o", p=128)
            )
            nc.gpsimd.dma_start(out=bv_sb, in_=bvd.rearrange("(o e) -> o e", o=1))

            def proj_k_row(c):
                for tb in range(4):
                    proj_qk_group(
                        ktf[:, :, tb * 512 : (tb + 1) * 512], kpT, bk_sb, wkT, c, tb
                    )

            proj_k_row(0)
            proj_qk_chunk(qc, qpT, bq_sb, wqT, 0)
            nc.gpsimd.dma_start(
                out=woT, in_=wotd.rearrange("(c p) o -> p c o", p=128)
            )

            # ---------------- attention ----------------------------------
            # normalized ctx, natural layout, per t-chunk of 128:
            # ctxn[t', e_local] for t = tb*512 + tc*128 + t'
            # assembled across the 4 j-units of a t-block.
            ctxn_sb = {}

            v_loaded = [None]

            def emit_unit(j, tb, interleave_v=False):
                ctx = ctx_ps.tile([128, 2, 512], F32, tag="ctx")
                # The 8 (head, t-chunk) PV accumulation regions interleave
                # within this tile's two banks.  A start=True on hardware
                # clears has_written for the WHOLE bank, wiping sibling
                # regions, so instead: zero the tile explicitly and run every
                # PV matmul in accumulate mode (correct for any has_written
                # state: bit=1 accumulates onto 0, bit=0 overwrites 0).
                nc.vector.memset(ctx, 0.0)
                tsl = slice(tb * 512, (tb + 1) * 512)
                for s in range(16):
                    if interleave_v:
                        # v-proj rides inside the first unit's s-loop so PE
                        # fills the ACT-bound gaps; chunk DMA every 4 s.
                        if s % 4 == 0:
                            v_loaded[0] = load_x_chunk(vtd, s // 4)
                        proj_v_schunk(v_loaded[0], s)
                    sc = sc_ps.tile([128, 1024], F32, tag="sc")
                    nc.tensor.matmul(
                        sc[:, 0:512],
                        lhsT=kpT[0:64, j, s * 128 : (s + 1) * 128],
                        rhs=qpT[0:64, j, tsl],
                        start=True,
                        stop=True,
                    )
                    nc.tensor.matmul(
                        sc[:, 512:1024],
                        lhsT=kpT[64:128, j, s * 128 : (s + 1) * 128],
                        rhs=qpT[64:128, j, tsl],
                        start=True,
                        stop=True,
                    )
                    pt = ptp.tile([128, 1024], BF16, tag="pt", bufs=8)
                    nc.scalar.activation(
                        out=pt,
                        in_=sc,
                        func=mybir.ActivationFunctionType.Exp,
                        scale=0.125,
                    )
                    for hh in range(2):
                        for tc in range(4):
                            nc.tensor.matmul(
                                ctx[:, hh, tc * 128 : tc * 128 + 65],
                                lhsT=pt[
                                    :, hh * 512 + tc * 128 : hh * 512 + (tc + 1) * 128
                                ],
                                rhs=vp_ext[
                                    :, s, (2 * j + hh) * 65 : (2 * j + hh + 1) * 65
                                ],
                                start=False,
                                stop=(s == 15),
                                skip_group_check=True,
                            )
                # normalize: column tc*128+64 of ctx[:, hh, :] holds denoms
                recip = normp.tile([128, 2, 4, 1], F32, tag="recip", bufs=2)
                nc.vector.reciprocal(
                    out=recip,
                    in_=ctx.rearrange("p h (tc x) -> p h tc x", x=128)[:, :, :, 64:65],
                )
                for tc in range(4):
                    for hh in range(2):
                        hl = 2 * j + hh
                        nc.vector.tensor_scalar_mul(
                            out=ctxn_sb[tb, tc][:, hl * 64 : (hl + 1) * 64],
                            in0=ctx[:, hh, tc * 128 : tc * 128 + 64],
                            scalar1=recip[:, hh, tc, :],
                        )

            def emit_transposes_j(tb, j, ctxnT):
                """Transpose the e-columns of head-pair j (128 of 512) of all
                four t-chunks of tb into ctxnT[:, j, :] — runs right after
                unit (j, tb)'s normalize, keeping the tail off the end."""
                tr = proj_ps.tile([128, 512], BF16, tag="proj")
                for tc in range(4):
                    nc.tensor.transpose(
                        tr[:, tc * 128 : (tc + 1) * 128],
                        ctxn_sb[tb, tc][:, j * 128 : (j + 1) * 128],
                        ident,
                    )
                nc.vector.tensor_copy(out=ctxnT[:, j, :], in_=tr)

            def emit_out_proj(tb, ctxnT):
                tsl = slice(tb * 512, (tb + 1) * 512)
                for o in range(8):
                    ps = proj_ps.tile([128, 512], F32, tag="proj")
                    for c in range(4):
                        nc.tensor.matmul(
                            ps,
                            lhsT=woT[:, c, o * 128 : (o + 1) * 128],
                            rhs=ctxnT[:, c, :],
                            start=(c == 0),
                            stop=(c == 3),
                        )
                    osb = osbp.tile([128, 512], F32, tag="osb")
                    nc.vector.tensor_copy(out=osb, in_=ps)
                    nc.sync.dma_start(
                        out=outd[o * 128 : (o + 1) * 128, tsl], in_=osb
                    )

            # Per-t-block flow: 4 attention units (one per head pair), each
            # followed by its slice of the ctx transpose.  The output
            # projection of t-block tb-1 is emitted INSIDE t-block tb's units
            # so the (ACT-bound) attention keeps streaming while PE absorbs
            # it in idle gaps.  k-projection rows 1..3 ride inside tb0.
            ctxnT = {}
            for tb in range(4):
                for tc in range(4):
                    ctxn_sb[tb, tc] = normp.tile(
                        [128, EL], BF16, tag="ctxn", name=f"ctxn{tb}_{tc}"
                    )
                ctxnT[tb] = normp.tile(
                    [128, 4, 512], BF16, tag="ctxnT", bufs=2, name=f"ctxnT{tb}"
                )
                for j in range(4):
                    if tb == 0 and j < 3:
                        proj_k_row(j + 1)  # row j+1 before unit j+1 needs it
                    emit_unit(j, tb, interleave_v=(tb == 0 and j == 0))
                    # PE work that consumes FRESH DVE output (the ctx
                    # transposes) is emitted a full t-block later: engine
                    # queues run in the static scheduled order, so putting it
                    # here would stall PE on unit j's normalize.
                    if j == 0 and tb > 0:
                        for jj in range(4):
                            emit_transposes_j(tb - 1, jj, ctxnT[tb - 1])
                    if j == 1 and tb > 0:
                        emit_out_proj(tb - 1, ctxnT[tb - 1])
                    # prefetch next t-block's q projection into the ACT-bound
                    # attention phase
                    if j == 1 and tb < 3:
                        qc = load_x_chunk(qtd, tb + 1)
                    if j == 2 and tb < 3:
                        proj_qk_chunk(qc, qpT, bq_sb, wqT, tb + 1)

            for jj in range(4):
                emit_transposes_j(3, jj, ctxnT[3])
            emit_out_proj(3, ctxnT[3])

    legalize_waits(nc)
    return nc


def _make_in_maps(inputs):
    import ml_dtypes

    bf16 = ml_dtypes.bfloat16
    q, k, v = inputs["q"], inputs["k"], inputs["v"]
    f32 = np.float32

    def tcast(a):
        return np.ascontiguousarray(np.asarray(a, dtype=f32).T.astype(bf16))

    # per-batch transposed activations, shared by the two cores of a batch
    qT = [tcast(q[b]) for b in range(B)]
    kT = [tcast(k[b]) for b in range(B)]
    vT = [tcast(v[b]) for b in range(B)]
    Wq, Wk, Wv, Wo = inputs["Wq"], inputs["Wk"], inputs["Wv"], inputs["Wo"]
    in_maps = []
    for c in range(N_CORES):
        b, hh = c // 2, c % 2
        esl = slice(hh * EL, (hh + 1) * EL)
        in_maps.append(
            {
                "qT": qT[b],
                "kT": kT[b],
                "vT": vT[b],
                "wqT": tcast(np.asarray(Wq, dtype=f32)[esl]),
                "wkT": tcast(np.asarray(Wk, dtype=f32)[esl]),
                "wvT": tcast(np.asarray(Wv, dtype=f32)[esl]),
                "woT": tcast(np.asarray(Wo, dtype=f32)[:, esl]),
                "bq": np.ascontiguousarray(inputs["bq"][esl], dtype=f32),
                "bk": np.ascontiguousarray(inputs["bk"][esl], dtype=f32),
                "bv": np.ascontiguousarray(inputs["bv"][esl], dtype=f32),
            }
        )
    return in_maps


def _gather(results, bo):
    out = np.empty((B, T, E), dtype=np.float32)
    for b in range(B):
        acc = results[2 * b]["outT"].T + results[2 * b + 1]["outT"].T
        out[b] = acc + bo[None, :]
    return out


def run(inputs, **spmd_kwargs):
    if "nc" not in _CACHED:
        _CACHED["nc"] = build_program()
    nc = _CACHED["nc"]
    in_maps = _make_in_maps(inputs)
    res = run_bass_kernel_spmd(nc, in_maps, core_ids=list(range(N_CORES)), **spmd_kwargs)
    out = _gather(res.results, np.asarray(inputs["bo"], dtype=np.float32))
    return out, res


def kernel(**inputs) -> np.ndarray:
    out, _ = run(inputs)
    return out


# revision 40
# speedup vs baseline: 1.2970x; 1.0203x over previous
"""Multi-head attention (B=4, T=S=2048, E=1024, H=16) on 8 trn2 NeuronCores.

Sharding: core c handles batch b = c // 2 and head-half hh = c % 2
(8 of 16 heads).  Each core computes its heads' Q/K/V projections,
attention, and a partial output projection (contraction over its 512
e-dims).  The host sums the two partial outputs per batch and adds bo.

Key layout choices (v2):
 - The host passes PRE-TRANSPOSED activations and weights (q.T, k.T,
   v.T, Wq_slice.T, ...), so no on-chip transposes of x or W are
   needed: DMA loads land directly in the [e_in partitions, t] layout
   the projections consume (f32->bf16 cast in the SWDGE DMA).
 - scores.T = kp @ qp.T is computed per head as [s, t] tiles
   (s on partitions), exp'd on ACT into bf16 pt tiles.
 - PV runs in the "natural" orientation: ctx[t, hd] = sum_s
   pt[s, t] * vp[s, hd], i.e. lhsT = pt (stationary), rhs = vp
   with an extra ones-column producing the softmax denominator in
   column 64.  Output columns per matmul are 65 instead of 512,
   which is ~2x fewer PE cycles for the PV stage.
 - Normalization is a per-partition (per-t) reciprocal multiply on
   DVE, no cross-partition broadcast needed.
 - ctx is transposed back (PE transposes) only for the tiny
   [2048 x 512] normalized context, feeding the output projection.
"""

import numpy as np

import concourse.bass as bass
import concourse.mybir as mybir
import concourse.tile as tile
from concourse.bass_utils import run_bass_kernel_spmd
from concourse.masks import make_identity

F32 = mybir.dt.float32
BF16 = mybir.dt.bfloat16

B, T, E = 4, 2048, 1024
H = 16  # global heads
HL = 8  # heads per core (local)
HD = 64  # head dim
EL = HL * HD  # 512, e-dims per core
N_CORES = 8

_CACHED = {}


def legalize_waits(nc, cap=1):
    """Hoist semaphore waits so no instruction carries more than `cap`.

    The cayman 64B ISA instruction format has a single wait slot
    (NEURON_ISA_TPB_EVENTS); this container's walrus rejects instructions
    with more attached waits ("Too many sync wait commands").  Tile's sem
    assignment freely attaches several, so we split the excess onto
    standalone InstEventSemaphore carriers (exactly what raw-bass
    wait_ge emits) on the same engine, immediately before.
    """
    import bass_rust

    # Pass 1: statically-known final value of every semaphore (sum of all
    # attached increments) — needed to replace the tail RANGE_CLEAR (an
    # InstISA opcode this walrus can't codegen) with sem-dec updates.
    totals = {}
    names = {}
    for f in nc.m.functions:
        for bb in f.blocks:
            for ins in bb.instructions:
                si = ins.sync_info
                if si is None:
                    continue
                for u in si.on_update or []:
                    if u.sync_type == "semaphore":
                        sign = 1 if u.update_mode in ("sem-inc", "sem-add-imm") else -1
                        totals[u.id] = totals.get(u.id, 0) + sign * u.update_value
                        names[u.id] = u.ant_name

    n = 0
    for f in nc.m.functions:
        for bb in f.blocks:
            insts = bb.instructions
            out = []
            changed = False
            for ins in insts:
                if type(ins).__name__ == "InstISA" and "RANGE_CLEAR" in str(ins):
                    import re

                    m = re.search(r"range_first=(\d+) range_last=(\d+)", str(ins))
                    first, last = int(m.group(1)), int(m.group(2))
                    for sid in range(first, last + 1):
                        tot = totals.get(sid, 0)
                        if tot == 0:
                            continue
                        ev = mybir.InstEventSemaphore(name=f"I-LC{n}", ins=[], outs=[])
                        n += 1
                        ev.engine = ins.engine
                        ev.sync_info = bass_rust.SyncInfo(
                            on_wait=[],
                            on_update=[
                                bass_rust.SyncUpdate(
                                    sync_type="semaphore",
                                    id=sid,
                                    ant_name=names.get(sid, f"sem{sid}"),
                                    update_mode="sem-sub-imm",
                                    update_value=tot,
                                    update_reg=None,
                                )
                            ],
                        )
                        out.append(ev)
                    changed = True
                    continue
                si = ins.sync_info
                ws = list(si.on_wait) if (si is not None and si.on_wait) else []
                if len(ws) > cap:
                    for w in ws[: len(ws) - cap]:
                        ev = mybir.InstEventSemaphore(
                            name=f"I-LW{n}", ins=[], outs=[]
                        )
                        n += 1
                        ev.engine = ins.engine
                        ev.sync_info = bass_rust.SyncInfo(
                            on_wait=[w], on_update=[]
                        )
                        out.append(ev)
                    si.on_wait = ws[len(ws) - cap :]
                    changed = True
                out.append(ins)
            if changed:
                insts[:] = out
    return n


def build_program():
    nc = bass.Bass()

    # Activations/weights arrive pre-transposed AND pre-cast to bf16 on the
    # host: halves the DMA bytes and removes the cast from the DMA path.
    qtd = nc.declare_dram_parameter("qT", [E, T], BF16, isOutput=False)
    ktd = nc.declare_dram_parameter("kT", [E, T], BF16, isOutput=False)
    vtd = nc.declare_dram_parameter("vT", [E, T], BF16, isOutput=False)
    wqtd = nc.declare_dram_parameter("wqT", [E, EL], BF16, isOutput=False)
    wktd = nc.declare_dram_parameter("wkT", [E, EL], BF16, isOutput=False)
    wvtd = nc.declare_dram_parameter("wvT", [E, EL], BF16, isOutput=False)
    wotd = nc.declare_dram_parameter("woT", [EL, E], BF16, isOutput=False)
    bqd = nc.declare_dram_parameter("bq", [EL], F32, isOutput=False)
    bkd = nc.declare_dram_parameter("bk", [EL], F32, isOutput=False)
    bvd = nc.declare_dram_parameter("bv", [EL], F32, isOutput=False)
    outd = nc.declare_dram_parameter("outT", [E, T], F32, isOutput=True)

    with tile.TileContext(nc, pool_alloc_mode="queue") as tc:
        with (
            tc.tile_pool(name="singles", bufs=1) as singles,
            tc.tile_pool(name="xin", bufs=4) as xin,
            tc.tile_pool(name="acts", bufs=1) as acts,
            tc.tile_pool(name="pt", bufs=1) as ptp,
            tc.tile_pool(name="norm", bufs=8) as normp,
            tc.tile_pool(name="osb", bufs=4) as osbp,
            tc.tile_pool(name="proj_ps", bufs=2, space="PSUM") as proj_ps,
            tc.tile_pool(name="sc_ps", bufs=2, space="PSUM") as sc_ps,
            tc.tile_pool(name="ctx_ps", bufs=1, space="PSUM") as ctx_ps,
        ):
            # ---------------- prologue: weights / biases / consts ----------
            ident = singles.tile([128, 128], BF16)
            make_identity(nc, ident)

            # Transposed bf16 weights, loaded directly (host pre-transposed):
            #   wqT[p, c, o] = Wq_c[o, c*128 + p]   (c,p) = e_in in [0,1024)
            wqT = singles.tile([128, 8, EL], BF16)
            wkT = singles.tile([128, 8, EL], BF16)
            wvT = singles.tile([128, 8, EL], BF16)
            # woT[p, c, o] = Wo_c[o, c*128 + p]     (c,p) = local e in [0,512)
            woT = singles.tile([128, 4, E], BF16)

            # k-path DMAs first: the whole k projection gates attention.
            nc.gpsimd.dma_start(
                out=wkT, in_=wktd.rearrange("(c p) o -> p c o", p=128)
            )
            bq_sb = singles.tile([128, 4], F32)
            bk_sb = singles.tile([128, 4], F32)
            nc.gpsimd.dma_start(out=bk_sb, in_=bkd.rearrange("(c p) -> p c", p=128))
            nc.gpsimd.dma_start(out=bq_sb, in_=bqd.rearrange("(c p) -> p c", p=128))
            ones_col = singles.tile([1, 128], BF16)
            nc.vector.memset(ones_col, 1.0)
            zero_row = singles.tile([1, 512], BF16)
            nc.vector.memset(zero_row, 0.0)
            bv_sb = singles.tile([1, EL], BF16)

            # ---------------- activations / projections --------------------
            # qpT[p, j, t] = qp[t, j*128 + p]  (pair j: head 2j at p<64)
            qpT = acts.tile([128, 4, T], BF16)
            kpT = acts.tile([128, 4, T], BF16)
            # vp_ext[p, s, h*65 + d] = vp[s*128 + p, h*64 + d]; col h*65+64 = 1
            vp_ext = acts.tile([128, 16, HL * 65], BF16)

            def load_x_chunk(xd, tb):
                """x.T [E, T] bf16 DRAM chunk tb -> bf16 SBUF [128, 8, 512]."""
                xc = xin.tile([128, 8, 512], BF16, tag="xin")
                nc.gpsimd.dma_start(
                    out=xc,
                    in_=xd[:, tb * 512 : (tb + 1) * 512].rearrange(
                        "(c p) t -> p c t", p=128
                    ),
                )
                return xc

            def proj_qk_group(xc, xpT, b_sb, wT, c, tb):
                """One (e_out chunk c, t-block tb) projection psum group."""
                ps = proj_ps.tile([128, 512], F32, tag="proj")
                for e in range(8):
                    nc.tensor.matmul(
                        ps,
                        lhsT=wT[:, e, c * 128 : (c + 1) * 128],
                        rhs=xc[:, e, :],
                        start=(e == 0),
                        stop=(e == 7),
                    )
                nc.vector.tensor_scalar_add(
                    out=xpT[:, c, tb * 512 : (tb + 1) * 512],
                    in0=ps,
                    scalar1=b_sb[:, c : c + 1],
                )

            def proj_qk_chunk(xc, xpT, b_sb, wT, tb):
                """One t-block of the q/k projection: fills xpT[:, :, tsl]."""
                for c in range(4):
                    proj_qk_group(xc, xpT, b_sb, wT, c, tb)

            def proj_v_schunk(vc, s):
                """One 128-row s-chunk of the v projection -> vp_ext[:, s, :].

                vc is the loaded vT chunk covering s; u = s % 4 indexes the
                128-slice within it.
                """
                u = s % 4
                ps = proj_ps.tile([128, 512], F32, tag="proj")
                for e in range(8):
                    nc.tensor.matmul(
                        ps,
                        lhsT=vc[:, e, u * 128 : (u + 1) * 128],
                        rhs=wvT[:, e, :],
                        start=(e == 0),
                        stop=False,
                    )
                # += ones ⊗ bv  (bias along the free dim)
                nc.tensor.matmul(ps, lhsT=ones_col, rhs=bv_sb, start=False, stop=True)
                nc.vector.memset(vp_ext[:, s, :], 1.0)
                nc.vector.tensor_copy(
                    out=vp_ext[:, s, :].rearrange("p (h x) -> p h x", x=65)[
                        :, :, 0:64
                    ],
                    in_=ps.rearrange("p (h d) -> p h d", d=64),
                )

            # k is projected c-row-wise (c = head-pair j): attention unit
            # (j, tb) only needs row c=j of kpT, so unit (0, 0) can start
            # right after row 0 — rows 1..3 are spread into tb0's units.
            # kT is loaded as one full tile (each c-row contracts over all T).
            ktf = xin.tile([128, 8, T], BF16, tag="ktf", bufs=1)
            nc.gpsimd.dma_start(
                out=ktf[:, :, 0:512],
                in_=ktd[:, 0:512].rearrange("(c p) t -> p c t", p=128),
            )
            nc.gpsimd.dma_start(
                out=ktf[:, :, 512:1024],
                in_=ktd[:, 512:1024].rearrange("(c p) t -> p c t", p=128),
            )
            nc.gpsimd.dma_start(
                out=wqT, in_=wqtd.rearrange("(c p) o -> p c o", p=128)
            )
            qc = load_x_chunk(qtd, 0)
            nc.gpsimd.dma_start(
                out=ktf[:, :, 1024:1536],
                in_=ktd[:, 1024:1536].rearrange("(c p) t -> p c t", p=128),
            )
            nc.gpsimd.dma_start(
                out=ktf[:, :, 1536:2048],
                in_=ktd[:, 1536:2048].rearrange("(c p) t -> p c t", p=128),
            )
            nc.gpsimd.dma_start(
                out=wvT, in_=wvtd.rearrange("(c p) o -> p c o", p=128)
            )
            nc.gpsimd.dma_start(out=bv_sb, in_=bvd.rearrange("(o e) -> o e", o=1))
            # v chunks: needed from window 1 of the tb0 pipeline (~30us in)
            vcs = [load_x_chunk(vtd, i) for i in range(4)]

            # Minimal pre-attention PE work: one k-group and one q-group —
            # exactly what scores (j=0, tb=0, s-chunk 0) needs.  Everything
            # else rides just-in-time inside tb0's units.
            proj_qk_group(ktf[:, :, 0:512], kpT, bk_sb, wkT, 0, 0)
            proj_qk_group(qc, qpT, bq_sb, wqT, 0, 0)
            nc.gpsimd.dma_start(
                out=woT, in_=wotd.rearrange("(c p) o -> p c o", p=128)
            )

            def kg_thunk(c, tbk):
                def f():
                    proj_qk_group(
                        ktf[:, :, tbk * 512 : (tbk + 1) * 512],
                        kpT, bk_sb, wkT, c, tbk,
                    )
                return f

            def qg_thunk(c):
                def f():
                    proj_qk_group(qc, qpT, bq_sb, wqT, c, 0)
                return f

            # ---------------- attention ----------------------------------
            # normalized ctx, natural layout, per t-chunk of 128:
            # ctxn[t', e_local] for t = tb*512 + tc*128 + t'
            # assembled across the 4 j-units of a t-block.
            ctxn_sb = {}

            v_loaded = [None]

            pt_store = {}  # u -> list of 16 pt tiles
            ctx_store = {}  # u -> ctx psum tile

            def emit_zero(u):
                """Alloc unit u's ctx tile and zero it.  The 8 (head,
                t-chunk) PV accumulation regions interleave within its two
                banks; a start=True on hardware clears has_written for the
                WHOLE bank, wiping sibling regions, so instead: zero each
                bank with a single-group ones⊗zero matmul and run every PV
                matmul in accumulate mode (correct for any has_written
                state: bit=1 accumulates onto 0, bit=0 overwrites)."""
                ctx = ctx_ps.tile([128, 2, 512], F32, tag="ctx", name=f"ctx{u}")
                ctx_store[u] = ctx
                for hh in range(2):
                    nc.tensor.matmul(
                        ctx[:, hh, :], lhsT=ones_col, rhs=zero_row,
                        start=True, stop=True,
                    )

            def emit_scores(u, s):
                tb, j = divmod(u, 4)
                tsl = slice(tb * 512, (tb + 1) * 512)
                sc = sc_ps.tile([128, 1024], F32, tag="sc")
                nc.tensor.matmul(
                    sc[:, 0:512],
                    lhsT=kpT[0:64, j, s * 128 : (s + 1) * 128],
                    rhs=qpT[0:64, j, tsl],
                    start=True,
                    stop=True,
                )
                nc.tensor.matmul(
                    sc[:, 512:1024],
                    lhsT=kpT[64:128, j, s * 128 : (s + 1) * 128],
                    rhs=qpT[64:128, j, tsl],
                    start=True,
                    stop=True,
                )
                pt = ptp.tile(
                    [128, 1024], BF16, tag="pt", bufs=17, name=f"pt{u}_{s}"
                )
                nc.scalar.activation(
                    out=pt,
                    in_=sc,
                    func=mybir.ActivationFunctionType.Exp,
                    scale=0.125,
                )
                pt_store.setdefault(u, {})[s] = pt

            def emit_pv(u, s):
                tb, j = divmod(u, 4)
                ctx = ctx_store[u]
                pt = pt_store[u].pop(s)
                for hh in range(2):
                    for tc in range(4):
                        nc.tensor.matmul(
                            ctx[:, hh, tc * 128 : tc * 128 + 65],
                            lhsT=pt[
                                :, hh * 512 + tc * 128 : hh * 512 + (tc + 1) * 128
                            ],
                            rhs=vp_ext[
                                :, s, (2 * j + hh) * 65 : (2 * j + hh + 1) * 65
                            ],
                            start=False,
                            stop=(s == 15),
                            skip_group_check=True,
                        )

            def emit_norm(u):
                # normalize: column tc*128+64 of ctx[:, hh, :] holds denoms
                tb, j = divmod(u, 4)
                ctx = ctx_store.pop(u)
                recip = normp.tile(
                    [128, 2, 4, 1], F32, tag="recip", bufs=2, name=f"recip{u}"
                )
                nc.vector.reciprocal(
                    out=recip,
                    in_=ctx.rearrange("p h (tc x) -> p h tc x", x=128)[
                        :, :, :, 64:65
                    ],
                )
                for tc in range(4):
                    for hh in range(2):
                        hl = 2 * j + hh
                        nc.vector.tensor_scalar_mul(
                            out=ctxn_sb[tb, tc][:, hl * 64 : (hl + 1) * 64],
                            in0=ctx[:, hh, tc * 128 : tc * 128 + 64],
                            scalar1=recip[:, hh, tc, :],
                        )

            def emit_unit(j, tb, fills=()):
                """Same-unit scores+PV streaming (steady-state structure)."""
                u = 4 * tb + j
                emit_zero(u)
                fq = list(fills)
                for s in range(16):
                    emit_scores(u, s)
                    if fq and s % 3 == 2:
                        fq.pop(0)()
                    emit_pv(u, s)
                emit_norm(u)

            def emit_transposes_j(tb, j, ctxnT):
                """Transpose the e-columns of head-pair j (128 of 512) of all
                four t-chunks of tb into ctxnT[:, j, :] — runs right after
                unit (j, tb)'s normalize, keeping the tail off the end."""
                tr = proj_ps.tile([128, 512], BF16, tag="proj")
                for tc in range(4):
                    nc.tensor.transpose(
                        tr[:, tc * 128 : (tc + 1) * 128],
                        ctxn_sb[tb, tc][:, j * 128 : (j + 1) * 128],
                        ident,
                    )
                nc.vector.tensor_copy(out=ctxnT[:, j, :], in_=tr)

            def emit_out_proj(tb, ctxnT):
                tsl = slice(tb * 512, (tb + 1) * 512)
                for o in range(8):
                    ps = proj_ps.tile([128, 512], F32, tag="proj")
                    for c in range(4):
                        nc.tensor.matmul(
                            ps,
                            lhsT=woT[:, c, o * 128 : (o + 1) * 128],
                            rhs=ctxnT[:, c, :],
                            start=(c == 0),
                            stop=(c == 3),
                        )
                    osb = osbp.tile([128, 512], F32, tag="osb")
                    nc.vector.tensor_copy(out=osb, in_=ps)
                    nc.sync.dma_start(
                        out=outd[o * 128 : (o + 1) * 128, tsl], in_=osb
                    )

            ctxnT = {}
            for tb in range(4):
                for tc in range(4):
                    ctxn_sb[tb, tc] = normp.tile(
                        [128, EL], BF16, tag="ctxn", name=f"ctxn{tb}_{tc}"
                    )
                ctxnT[tb] = normp.tile(
                    [128, 4, 512], BF16, tag="ctxnT", bufs=2, name=f"ctxnT{tb}"
                )

            # ---- tb0: one-behind-PV pipeline over 5 windows --------------
            # Window w streams scores/exp of unit w while PV of unit w-1
            # absorbs the v-projection (w=1) and k-row JIT fills; a short
            # PV-only flush window closes the block.  This keeps ACT (exp)
            # streaming from ~15us even though PE has ~100us of projection
            # work to retire in tb0.
            qc1 = [None]
            vc_pre = [None]

            def load_qc1():
                qc1[0] = load_x_chunk(qtd, 1)

            for w in range(5):
                u_sc = w if w < 4 else None
                u_pv = w - 1 if w >= 1 else None
                fills = []
                if w < 3:
                    fills.append(qg_thunk(w + 1))
                if w == 3:
                    fills.append(load_qc1)
                if w == 4:
                    for c in range(4):
                        fills.append(
                            (lambda c=c: proj_qk_group(
                                qc1[0], qpT, bq_sb, wqT, c, 1
                            ))
                        )
                if u_pv is not None:
                    emit_zero(u_pv)
                for s in range(16):
                    if u_sc is not None:
                        if s % 4 == 0 and (w > 0 or s > 0):
                            # k-row c=w, t-chunk s//4 just-in-time before the
                            # scores that consume it
                            proj_qk_group(
                                ktf[:, :, (s // 4) * 512 : (s // 4 + 1) * 512],
                                kpT, bk_sb, wkT, w, s // 4,
                            )
                        emit_scores(u_sc, s)
                    if w == 1:
                        proj_v_schunk(vcs[s // 4], s)
                    if u_pv is not None:
                        emit_pv(u_pv, s)
                    if fills and s % 3 == 2:
                        fills.pop(0)()
                for f in fills:
                    f()
                if u_pv is not None:
                    emit_norm(u_pv)

            # ---- tb1..3: steady state, same-unit PV ----------------------
            qc_t = qc1[0]
            for tb in range(1, 4):
                for j in range(4):
                    emit_unit(j, tb)
                    # PE work that consumes FRESH DVE output (the ctx
                    # transposes) is emitted a full t-block later: engine
                    # queues run in the static scheduled order, so putting it
                    # here would stall PE on unit j's normalize.
                    if j == 0:
                        for jj in range(4):
                            emit_transposes_j(tb - 1, jj, ctxnT[tb - 1])
                    if j == 1:
                        emit_out_proj(tb - 1, ctxnT[tb - 1])
                    # prefetch next t-block's q projection into the ACT-bound
                    # attention phase
                    if j == 1 and tb < 3:
                        qc_t = load_x_chunk(qtd, tb + 1)
                    if j == 2 and tb < 3:
                        proj_qk_chunk(qc_t, qpT, bq_sb, wqT, tb + 1)

            for jj in range(4):
                emit_transposes_j(3, jj, ctxnT[3])
            emit_out_proj(3, ctxnT[3])

    legalize_waits(nc)
    return nc


def _make_in_maps(inputs):
    import ml_dtypes

    bf16 = ml_dtypes.bfloat16
    q, k, v = inputs["q"], inputs["k"], inputs["v"]
    f32 = np.float32

    def tcast(a):
        return np.ascontiguousarray(np.asarray(a, dtype=f32).T.astype(bf16))

    # per-batch transposed activations, shared by the two cores of a batch
    qT = [tcast(q[b]) for b in range(B)]
    kT = [tcast(k[b]) for b in range(B)]
    vT = [tcast(v[b]) for b in range(B)]
    Wq, Wk, Wv, Wo = inputs["Wq"], inputs["Wk"], inputs["Wv"], inputs["Wo"]
    in_maps = []
    for c in range(N_CORES):
        b, hh = c // 2, c % 2
        esl = slice(hh * EL, (hh + 1) * EL)
        in_maps.append(
            {
                "qT": qT[b],
                "kT": kT[b],
                "vT": vT[b],
                "wqT": tcast(np.asarray(Wq, dtype=f32)[esl]),
                "wkT": tcast(np.asarray(Wk, dtype=f32)[esl]),
                "wvT": tcast(np.asarray(Wv, dtype=f32)[esl]),
                "woT": tcast(np.asarray(Wo, dtype=f32)[:, esl]),
                "bq": np.ascontiguousarray(inputs["bq"][esl], dtype=f32),
                "bk": np.ascontiguousarray(inputs["bk"][esl], dtype=f32),
                "bv": np.ascontiguousarray(inputs["bv"][esl], dtype=f32),
            }
        )
    return in_maps


def _gather(results, bo):
    out = np.empty((B, T, E), dtype=np.float32)
    for b in range(B):
        acc = results[2 * b]["outT"].T + results[2 * b + 1]["outT"].T
        out[b] = acc + bo[None, :]
    return out


def run(inputs, **spmd_kwargs):
    if "nc" not in _CACHED:
        _CACHED["nc"] = build_program()
    nc = _CACHED["nc"]
    in_maps = _make_in_maps(inputs)
    res = run_bass_kernel_spmd(nc, in_maps, core_ids=list(range(N_CORES)), **spmd_kwargs)
    out = _gather(res.results, np.asarray(inputs["bo"], dtype=np.float32))
    return out, res


def kernel(**inputs) -> np.ndarray:
    out, _ = run(inputs)
    return out
